# revision 42
# baseline (speedup 1.0000x reference)
"""Trainium2 Bass kernel for the DWN block:
LayerNorm -> LRU (complex diagonal scan) -> GELU -> Linear(d,2d) -> GLU -> +x.

V3 strategy (measured-HW-driven rework of the V1 fp16 baseline, ~133us):
- Data-parallel: 1 batch element per NeuronCore (8 cores), SPMD NEFF.
- Transposed on-device layout [feature, time]: every matmul contracts the
  partition axis; the LRU scan runs along the free axis via the twiddle
  decoupling (two real tensor_tensor_scans per chunk).
- All matmuls fp16.  (fp8/DoubleRow was measured: it only relieves PE,
  which is not the bottleneck, while its xhat8/h8 conversions tax DVE -
  the actual wall at ~85us: scans run at ~2.9 cyc/elem on HW.)
- LayerNorm prologue for all chunks using ONLY the reciprocal_sqrt_and_small
  act table (Copy/Square/Rsqrt): rstd in one Rsqrt op (no DVE reciprocal),
  x^2 on ScalarE.  The pipeline then uses gelu_and_others (Gelu/Tanh/Copy).
  Exactly two act-table loads.
- Every DVE op keeps flattened 2-D access patterns (3-D APs measurably lose
  the 2x 16-bit packing on HW).
- No GPSIMD compute (measured 3.3-3.9us per op + SBUF-port contention
  with DVE); GPSIMD does nothing, weight DMAs ride the sync queue.
- Input x loaded once as fp16; residual add in fp16; fp16 output
  (converted to fp32 on host).
- 4-stage pipeline interleaved across chunks; input DMAs ordered ahead of
  weight DMAs; per-chunk cos/sin tiles.
"""

import numpy as np

import concourse.bacc as bacc
import concourse.tile as tile
from concourse import mybir
from concourse import bass_utils

# ---- problem constants (hardcoded per contract) ----
B, L, D, S = 8, 2048, 512, 256
DFF = 2 * D
LN_EPS = 1e-5
N_CORES = 8

# ---- tiling ----
P = 128
TC = 512                 # time chunk
NCHUNK = L // TC         # 4
KD = D // P              # 4  k-tiles over d
KS = S // P              # 2  k-tiles over s
MD = D // P              # 4  m-tiles over d outputs

F32 = mybir.dt.float32
F16 = mybir.dt.float16
AOP = mybir.AluOpType
AF = mybir.ActivationFunctionType
NP16 = np.float16


def _pack_rb(r):
    """[P, KS, TC] broadcast decay, with r=0 at the fused-scan boundary
    (s-tile 1, t=0) so the 1024-wide scan resets there; the true carry is
    injected into data1 instead."""
    rb = np.broadcast_to(r.reshape(KS, P, 1), (KS, P, TC)).transpose(1, 0, 2).copy()
    rb[:, 1, 0] = 0.0
    return np.ascontiguousarray(rb).astype(np.float32)


def _pack_kpm(w, k_tiles, m):
    """[K, M] -> [128, k_tiles, M] host pack for lhsT storage (K = kt*128+p)."""
    K = k_tiles * P
    assert w.shape == (K, m)
    return np.ascontiguousarray(w.reshape(k_tiles, P, m).transpose(1, 0, 2))


def _build(nc, with_bc=False, with_ba=False):
    f32, f16 = F32, F16

    xT16 = nc.dram_tensor("xT16", [P, KD, L], f16, kind="ExternalInput")
    bt_re = nc.dram_tensor("bt_re", [P, KD, S], f16, kind="ExternalInput")
    bt_im = nc.dram_tensor("bt_im", [P, KD, S], f16, kind="ExternalInput")
    ct_re = nc.dram_tensor("ct_re", [P, KS, D], f16, kind="ExternalInput")
    ct_imn = nc.dram_tensor("ct_imn", [P, KS, D], f16, kind="ExternalInput")
    dt_w = nc.dram_tensor("dt_w", [P, KD, D], f16, kind="ExternalInput")
    wt = nc.dram_tensor("wt", [P, KD, DFF], f16, kind="ExternalInput")
    cosT = nc.dram_tensor("cosT", [P, KS, L], f16, kind="ExternalInput")
    sinT = nc.dram_tensor("sinT", [P, KS, L], f16, kind="ExternalInput")
    r_b = nc.dram_tensor("r_b", [P, KS, TC], f32, kind="ExternalInput")
    r_col = nc.dram_tensor("r_col", [P, KS], f32, kind="ExternalInput")
    bc_re = nc.dram_tensor("bc_re", [P, KS], f32, kind="ExternalInput")
    bc_im = nc.dram_tensor("bc_im", [P, KS], f32, kind="ExternalInput")
    gbias = nc.dram_tensor("gbias", [P, MD], f32, kind="ExternalInput")
    b_a = nc.dram_tensor("b_a", [P, MD], f32, kind="ExternalInput")
    b_gh = nc.dram_tensor("b_gh", [P, MD], f32, kind="ExternalInput")
    outT = nc.dram_tensor("outT", [P, KD, L], f16, kind="ExternalOutput")

    with tile.TileContext(nc) as tc:
        with (
            tc.tile_pool(name="wpool", bufs=1) as wpool,
            tc.tile_pool(name="io", bufs=2) as io,
            tc.tile_pool(name="work", bufs=1) as work,
            tc.tile_pool(name="carry", bufs=2) as carry_pool,
            tc.tile_pool(name="psum", bufs=1, space="PSUM") as psum,
        ):
            # inputs first (the LN prologue must never starve), then weights
            # in first-use order, cos/sin per chunk.
            x16s = []
            for ck in range(NCHUNK):
                x16 = io.tile([P, KD, TC], f16, tag="x16", bufs=4, name=f"x16_{ck}")
                nc.sync.dma_start(x16[:], xT16[:, :, ck * TC : (ck + 1) * TC])
                x16s.append(x16)

            w_bt_re = wpool.tile([P, KD, S], f16)
            nc.sync.dma_start(w_bt_re[:], bt_re[:])
            w_bt_im = wpool.tile([P, KD, S], f16)
            nc.sync.dma_start(w_bt_im[:], bt_im[:])
            w_r = wpool.tile([P, KS, TC], f32)
            nc.sync.dma_start(w_r[:], r_b[:])
            w_rcol = wpool.tile([P, KS], f32)
            nc.sync.dma_start(w_rcol[:], r_col[:])
            cos_sbs, sin_sbs = [], []
            for ck in range(NCHUNK):
                cs = io.tile([P, KS, TC], f16, tag="cos", bufs=4, name=f"cos_{ck}")
                nc.sync.dma_start(cs[:], cosT[:, :, ck * TC : (ck + 1) * TC])
                sn = io.tile([P, KS, TC], f16, tag="sin", bufs=4, name=f"sin_{ck}")
                nc.sync.dma_start(sn[:], sinT[:, :, ck * TC : (ck + 1) * TC])
                cos_sbs.append(cs)
                sin_sbs.append(sn)
                if ck == 0:
                    w_ct_re = wpool.tile([P, KS, D], f16)
                    nc.sync.dma_start(w_ct_re[:], ct_re[:])
                    w_ct_imn = wpool.tile([P, KS, D], f16)
                    nc.sync.dma_start(w_ct_imn[:], ct_imn[:])
                    w_dt = wpool.tile([P, KD, D], f16)
                    nc.sync.dma_start(w_dt[:], dt_w[:])
                if ck == 1:
                    w_wt = wpool.tile([P, KD, DFF], f16)
                    nc.sync.dma_start(w_wt[:], wt[:])
            w_bc_re = wpool.tile([P, KS], f32)
            nc.sync.dma_start(w_bc_re[:], bc_re[:])
            w_bc_im = wpool.tile([P, KS], f32)
            nc.sync.dma_start(w_bc_im[:], bc_im[:])
            w_gbias = wpool.tile([P, MD], f32)
            nc.sync.dma_start(w_gbias[:], gbias[:])
            w_ba = wpool.tile([P, MD], f32)
            nc.sync.dma_start(w_ba[:], b_a[:])
            w_bgh = wpool.tile([P, MD], f32)
            nc.sync.dma_start(w_bgh[:], b_gh[:])
            ones = wpool.tile([P, P], f16)
            nc.vector.memset(ones, 1.0)
            w_eps = wpool.tile([P, 1], f32)
            nc.vector.memset(w_eps, LN_EPS)
            w_one = wpool.tile([P, 1], f32)
            nc.vector.memset(w_one, 1.0)

            state = {}
            u_prev_ref = [None]
            fl = lambda t: t.rearrange("p s t -> p (s t)")
            fk = lambda t: t.rearrange("p k t -> p (k t)")

            # ---- prologue: LN stats (Copy/Square/Rsqrt table only) ----
            def prologue(ck):
                t0 = ck * TC
                x16 = x16s[ck]
                mu_ps = psum.tile([P, TC], f32, tag="pj", bufs=4, name=f"mu{ck}")
                for kt in range(KD):
                    nc.tensor.matmul(
                        mu_ps[:], lhsT=ones[:], rhs=x16[:, kt, :],
                        start=(kt == 0), stop=(kt == KD - 1),
                    )
                x2 = work.tile([P, KD, TC], f16, tag="x2", bufs=2, name=f"x2_{ck}")
                nc.scalar.activation(fk(x2), fk(x16), AF.Square)
                msq_ps = psum.tile([P, TC], f32, tag="pj", bufs=4, name=f"msq{ck}")
                for kt in range(KD):
                    nc.tensor.matmul(
                        msq_ps[:], lhsT=ones[:], rhs=x2[:, kt, :],
                        start=(kt == 0), stop=(kt == KD - 1),
                    )
                mu16 = work.tile([P, TC], f16, tag="mu16", bufs=4, name=f"m16_{ck}")
                nc.scalar.activation(mu16[:], mu_ps[:], AF.Copy, scale=1.0 / D)
                mu2 = work.tile([P, TC], f32, tag="mu2", bufs=2, name=f"mu2_{ck}")
                nc.scalar.activation(mu2[:], mu_ps[:], AF.Square, scale=1.0 / D)
                var = work.tile([P, TC], f32, tag="var", bufs=2, name=f"var_{ck}")
                nc.vector.scalar_tensor_tensor(
                    var[:], msq_ps[:], 1.0 / D, mu2[:],
                    op0=AOP.mult, op1=AOP.subtract,
                )
                sig = work.tile([P, TC], f32, tag="sig", bufs=2, name=f"sig_{ck}")
                nc.scalar.activation(sig[:], var[:], AF.Sqrt, bias=w_eps[:])
                r32 = work.tile([P, TC], f32, tag="r32", bufs=2, name=f"r32_{ck}")
                nc.vector.reciprocal_approx_fast(r32[:], sig[:])
                rstd = work.tile([P, TC], f16, tag="rstd", bufs=4, name=f"rs_{ck}")
                nc.scalar.activation(rstd[:], r32[:], AF.Copy)
                state[ck] = dict(x16=x16, mu16=mu16, rstd=rstd, t0=t0)

            # ---- stage 0b: xhat16 (per-kt ops: broadcast operands run 1x) ----
            def stage0b(ck):
                st_d = state[ck]
                x16, mu16, rstd = st_d["x16"], st_d["mu16"], st_d["rstd"]
                xc = work.tile([P, KD, TC], f16, tag="xc", bufs=2, name=f"xc_{ck}")
                xhat = work.tile([P, KD, TC], f16, tag="xh", bufs=4, name=f"xh_{ck}")
                for kt in range(KD):
                    nc.vector.tensor_sub(xc[:, kt, :], x16[:, kt, :], mu16[:])
                for kt in range(KD):
                    nc.vector.tensor_mul(xhat[:, kt, :], xc[:, kt, :], rstd[:])
                st_d["xhat"] = xhat

            # ---- stage 1: Bu matmuls + twiddle + scan ----
            gelu_warm = [False]

            def stage1(ck):
                st_d = state[ck]
                if not gelu_warm[0]:
                    # dummy gelu: hoists the gelu act-table load into the
                    # ScalarE idle window right after the LN prologue instead
                    # of the y(0)->gelu(0)->W(0) latency chain.
                    warm = work.tile([P, 1], f16, tag="warm", bufs=1)
                    nc.scalar.activation(warm[:], w_one[:], AF.Gelu)
                    gelu_warm[0] = True
                xhat = st_d["xhat"]
                cos_sb, sin_sb = cos_sbs[ck], sin_sbs[ck]

                ps_bu = [
                    [
                        psum.tile([P, TC], f32, tag="bu", bufs=2, name=f"bu{c}{st}_{ck}")
                        for st in range(KS)
                    ]
                    for c in range(2)
                ]
                for st in range(KS):
                    for comp, w_bt in ((0, w_bt_re), (1, w_bt_im)):
                        for kt in range(KD):
                            nc.tensor.matmul(
                                ps_bu[comp][st][:],
                                lhsT=w_bt[:, kt, st * P : (st + 1) * P],
                                rhs=xhat[:, kt, :],
                                start=(kt == 0),
                                stop=(kt == KD - 1),
                            )

                bu_re = work.tile([P, KS, TC], f16, tag="bu_re", bufs=2, name=f"bur_{ck}")
                bu_im = work.tile([P, KS, TC], f16, tag="bu_im", bufs=2, name=f"bui_{ck}")
                for st in range(KS):
                    for comp, bu_t, w_bc in (
                        (0, bu_re, w_bc_re), (1, bu_im, w_bc_im),
                    ):
                        if with_bc:
                            nc.vector.tensor_scalar_add(
                                bu_t[:, st, :], ps_bu[comp][st][:],
                                w_bc[:, st : st + 1],
                            )
                        else:
                            nc.scalar.activation(
                                bu_t[:, st, :], ps_bu[comp][st][:], AF.Copy,
                            )

                c_re = work.tile([P, KS, TC], f16, tag="c_re", bufs=2, name=f"cre_{ck}")
                c_im = work.tile([P, KS, TC], f16, tag="c_im", bufs=2, name=f"cim_{ck}")
                tw1 = work.tile([P, KS, TC], f16, tag="tw1", bufs=2, name=f"tw1_{ck}")
                tw2 = work.tile([P, KS, TC], f16, tag="tw2", bufs=2, name=f"tw2_{ck}")
                nc.vector.tensor_mul(fl(tw1), fl(cos_sb), fl(bu_re))
                nc.vector.tensor_mul(fl(tw2), fl(sin_sb), fl(bu_im))
                nc.vector.tensor_add(fl(c_re), fl(tw1), fl(tw2))
                nc.vector.tensor_mul(fl(tw1), fl(cos_sb), fl(bu_im))
                nc.vector.tensor_mul(fl(tw2), fl(sin_sb), fl(bu_re))
                nc.vector.tensor_sub(fl(c_im), fl(tw1), fl(tw2))

                u = carry_pool.tile([P, 2, KS, TC], f16, tag="u", bufs=3, name=f"u_{ck}")
                u_prev = u_prev_ref[0]
                for comp, c_t in ((0, c_re), (1, c_im)):
                    if u_prev is not None:
                        nc.vector.scalar_tensor_tensor(
                            c_t[:, 1, 0:1],
                            u_prev[:, comp, 1, TC - 1 : TC],
                            w_rcol[:, 1:2],
                            c_t[:, 1, 0:1],
                            op0=AOP.mult, op1=AOP.add,
                        )
                        init = u_prev[:, comp, 0, TC - 1 : TC]
                    else:
                        init = 0.0
                    nc.vector.tensor_tensor_scan(
                        u[:, comp, :, :].rearrange("p s t -> p (s t)"),
                        w_r.rearrange("p s t -> p (s t)"),
                        c_t.rearrange("p s t -> p (s t)"),
                        init,
                        op0=AOP.mult,
                        op1=AOP.add,
                    )
                u_prev_ref[0] = u
                state[ck]["u"] = u

            # ---- stage 2a: untwiddle (DVE only) ----
            def stage2a(ck):
                st_d = state[ck]
                u = st_d["u"]
                cos_sb, sin_sb = cos_sbs[ck], sin_sbs[ck]
                xs_re = work.tile([P, KS, TC], f16, tag="xs_re", bufs=3, name=f"xsr_{ck}")
                xs_im = work.tile([P, KS, TC], f16, tag="xs_im", bufs=3, name=f"xsi_{ck}")
                uw1 = work.tile([P, KS, TC], f16, tag="uw1", bufs=2, name=f"uw1_{ck}")
                uw2 = work.tile([P, KS, TC], f16, tag="uw2", bufs=2, name=f"uw2_{ck}")
                u_re = u[:, 0, :, :].rearrange("p s t -> p (s t)")
                u_im = u[:, 1, :, :].rearrange("p s t -> p (s t)")
                nc.vector.tensor_mul(fl(uw1), fl(cos_sb), u_re)
                nc.vector.tensor_mul(fl(uw2), fl(sin_sb), u_im)
                nc.vector.tensor_sub(fl(xs_re), fl(uw1), fl(uw2))
                nc.vector.tensor_mul(fl(uw1), fl(sin_sb), u_re)
                nc.vector.tensor_mul(fl(uw2), fl(cos_sb), u_im)
                nc.vector.tensor_add(fl(xs_im), fl(uw1), fl(uw2))
                st_d["xs_re"] = xs_re
                st_d["xs_im"] = xs_im

            # ---- stage 2b: y matmuls + gelu ----
            def stage2b(ck):
                st_d = state[ck]
                xhat = st_d["xhat"]
                xs_re, xs_im = st_d["xs_re"], st_d["xs_im"]

                h16 = work.tile([P, MD, TC], f16, tag="h16", bufs=3, name=f"h_{ck}")
                for mt in range(MD):
                    ps_y = psum.tile([P, TC], f32, tag="y", bufs=2, name=f"y{mt}_{ck}")
                    for kt in range(KD):
                        nc.tensor.matmul(
                            ps_y[:],
                            lhsT=w_dt[:, kt, mt * P : (mt + 1) * P],
                            rhs=xhat[:, kt, :],
                            start=(kt == 0), stop=False,
                        )
                    for st in range(KS):
                        nc.tensor.matmul(
                            ps_y[:],
                            lhsT=w_ct_re[:, st, mt * P : (mt + 1) * P],
                            rhs=xs_re[:, st, :],
                            start=False, stop=False,
                        )
                    for st in range(KS):
                        nc.tensor.matmul(
                            ps_y[:],
                            lhsT=w_ct_imn[:, st, mt * P : (mt + 1) * P],
                            rhs=xs_im[:, st, :],
                            start=False, stop=(st == KS - 1),
                        )
                    nc.scalar.activation(
                        h16[:, mt, :], ps_y[:], AF.Gelu,
                        bias=w_gbias[:, mt : mt + 1],
                    )
                state[ck]["h16"] = h16

            # ---- stage 3: W matmuls + tanh-GLU + residual + store ----
            def stage3(ck):
                st_d = state[ck]
                h16, x16, t0 = st_d["h16"], st_d["x16"], st_d["t0"]
                out_sb = io.tile([P, KD, TC], f16, tag="out", bufs=2, name=f"out_{ck}")
                q_all = work.tile([P, MD, TC], f16, tag="q_all", bufs=2, name=f"q_{ck}")
                for mt in range(MD):
                    ps_pa = psum.tile([P, TC], f32, tag="pj", bufs=4, name=f"pa{mt}_{ck}")
                    ps_pg = psum.tile([P, TC], f32, tag="pj", bufs=4, name=f"pg{mt}_{ck}")
                    for kt in range(KD):
                        nc.tensor.matmul(
                            ps_pa[:],
                            lhsT=w_wt[:, kt, mt * P : (mt + 1) * P],
                            rhs=h16[:, kt, :],
                            start=(kt == 0), stop=(kt == KD - 1),
                        )
                    for kt in range(KD):
                        nc.tensor.matmul(
                            ps_pg[:],
                            lhsT=w_wt[:, kt, D + mt * P : D + (mt + 1) * P],
                            rhs=h16[:, kt, :],
                            start=(kt == 0), stop=(kt == KD - 1),
                        )
                    th = work.tile([P, TC], f16, tag="th", bufs=2, name=f"th{mt}_{ck}")
                    nc.scalar.activation(
                        th[:], ps_pg[:], AF.Tanh,
                        bias=w_bgh[:, mt : mt + 1], scale=0.5,
                    )
                    a16 = work.tile([P, TC], f16, tag="a16", bufs=2, name=f"a16{mt}_{ck}")
                    if with_ba:
                        nc.scalar.activation(
                            a16[:], ps_pa[:], AF.Copy, bias=w_ba[:, mt : mt + 1],
                        )
                    else:
                        nc.scalar.activation(a16[:], ps_pa[:], AF.Copy)
                    w16 = work.tile([P, TC], f16, tag="w16", bufs=2, name=f"w16{mt}_{ck}")
                    nc.vector.tensor_scalar_add(w16[:], th[:], 1.0)
                    nc.vector.tensor_mul(q_all[:, mt, :], a16[:], w16[:])
                for half in range(2):
                    hd = half * (KD // 2)
                    nc.vector.tensor_add(
                        fk(out_sb[:, hd : hd + KD // 2, :]),
                        fk(q_all[:, hd : hd + KD // 2, :]),
                        fk(x16[:, hd : hd + KD // 2, :]),
                    )
                    nc.sync.dma_start(
                        outT[:, hd : hd + KD // 2, t0 : t0 + TC],
                        out_sb[:, hd : hd + KD // 2, :],
                    )
                del state[ck]

            seq = [("p", 0), ("p", 1), ("p", 2), ("p", 3)]
            order = [
                ("0b", 0), ("1", 0),
                ("0b", 1), ("2a", 0), ("2b", 0), ("1", 1),
                ("0b", 2), ("3", 0), ("2a", 1), ("2b", 1), ("1", 2),
                ("0b", 3), ("3", 1), ("2a", 2), ("2b", 2), ("1", 3),
                ("3", 2), ("2a", 3), ("2b", 3),
                ("3", 3),
            ]
            fns = {
                "p": prologue, "0b": stage0b, "1": stage1,
                "2a": stage2a, "2b": stage2b, "3": stage3,
            }
            for stg, ck in seq + order:
                fns[stg](ck)

    nc.compile()
    return nc


_NC_CACHE = {}


def _get_module(with_bc=False, with_ba=False, _scales=None):
    key = (with_bc, with_ba)
    if key not in _NC_CACHE:
        nc = bacc.Bacc("TRN2", target_bir_lowering=False, debug=False)
        _NC_CACHE[key] = _build(nc, with_bc=with_bc, with_ba=with_ba)
    return _NC_CACHE[key]


def _host_prepack(inputs):
    ln_w = np.asarray(inputs["ln_w"], np.float64)
    ln_b = np.asarray(inputs["ln_b"], np.float64)
    nu_log = np.asarray(inputs["nu_log"], np.float64)
    theta_log = np.asarray(inputs["theta_log"], np.float64)
    gamma_log = np.asarray(inputs["gamma_log"], np.float64)
    B_re = np.asarray(inputs["B_re"], np.float64)
    B_im = np.asarray(inputs["B_im"], np.float64)
    C_re = np.asarray(inputs["C_re"], np.float64)
    C_im = np.asarray(inputs["C_im"], np.float64)
    D_m = np.asarray(inputs["D"], np.float64)
    W_out = np.asarray(inputs["W_out"], np.float64)
    b_out = np.asarray(inputs["b_out"], np.float64)

    r = np.exp(-np.exp(nu_log))
    theta = np.exp(theta_log)
    g = np.exp(gamma_log)
    ang = theta[:, None] * np.arange(L, dtype=np.float64)[None, :]
    cos_t = np.cos(ang)
    sin_t = np.sin(ang)

    Bn_re = B_re * g[:, None]
    Bn_im = B_im * g[:, None]
    BnT_re = (Bn_re * ln_w[None, :]).T
    BnT_im = (Bn_im * ln_w[None, :]).T
    bc_re_v = Bn_re @ ln_b
    bc_im_v = Bn_im @ ln_b
    CT_re = C_re.T
    CT_imn = (-C_im).T
    DT = (D_m * ln_w[None, :]).T
    gbias_v = D_m @ ln_b
    WT = W_out.T.copy()
    WT[:, :D] *= 0.5
    b_a_v = 0.5 * b_out[:D]
    b_gh_v = 0.5 * b_out[D:]

    def cols(v, ntiles):
        return np.ascontiguousarray(np.asarray(v, np.float32).reshape(ntiles, P).T)

    return {
        "bt_re": _pack_kpm(BnT_re, KD, S).astype(NP16),
        "bt_im": _pack_kpm(BnT_im, KD, S).astype(NP16),
        "ct_re": _pack_kpm(CT_re, KS, D).astype(NP16),
        "ct_imn": _pack_kpm(CT_imn, KS, D).astype(NP16),
        "dt_w": _pack_kpm(DT, KD, D).astype(NP16),
        "wt": _pack_kpm(WT, KD, DFF).astype(NP16),
        "cosT": np.ascontiguousarray(
            cos_t.reshape(KS, P, L).transpose(1, 0, 2)
        ).astype(NP16),
        "sinT": np.ascontiguousarray(
            sin_t.reshape(KS, P, L).transpose(1, 0, 2)
        ).astype(NP16),
        "r_b": _pack_rb(r),
        "r_col": np.ascontiguousarray(r.reshape(KS, P).T).astype(np.float32),
        "bc_re": cols(bc_re_v, KS),
        "bc_im": cols(bc_im_v, KS),
        "gbias": cols(gbias_v, MD),
        "b_a": cols(b_a_v, MD),
        "b_gh": cols(b_gh_v, MD),
    }


def _make_in_maps(inputs):
    x = np.asarray(inputs["x"], np.float32)
    weights = _host_prepack(inputs)
    in_maps = []
    for b in range(B):
        xb = np.ascontiguousarray(x[b].T.reshape(KD, P, L).transpose(1, 0, 2))
        m = dict(weights)
        m["xT16"] = xb.astype(NP16)
        in_maps.append(m)
    return in_maps, ()


OUT_NAME = "outT"


def _unpack_out(ob):
    return np.asarray(ob, np.float32).transpose(1, 0, 2).reshape(D, L).T


def kernel(**inputs):
    in_maps, _ = _make_in_maps(inputs)
    with_bc = bool(np.any(np.asarray(inputs["ln_b"]) != 0))
    with_ba = bool(np.any(np.asarray(inputs["b_out"]) != 0))
    nc = _get_module(with_bc, with_ba)
    res = bass_utils.run_bass_kernel_spmd(nc, in_maps, core_ids=list(range(N_CORES)))
    out = np.empty((B, L, D), np.float32)
    for b in range(B):
        out[b] = _unpack_out(res.results[b][OUT_NAME])
    return out
